# revision 22
# baseline (speedup 1.0000x reference)
"""GQA attention block (B=2, S=2048, E=2048, H=32, HKV=8, D=64) on 8 trn2 cores.

Sharding: tensor-parallel over heads. Core c owns q-heads 4c..4c+3 and kv-head c.
Each core computes its heads' attention for ALL rows, then an AllToAll exchanges
head-blocks for row-blocks so each core runs the output projection for its own
512-row slice against the full out_w. Host concatenates row slices.

All matmuls run as float32r (TF32-like, full PE rate at N>=512). Softmax is
computed without max-subtraction (scores are O(4), exp cannot overflow), with
denominators obtained by augmenting V with a ones column.
"""

import numpy as np

B, S, E = 2, 2048, 2048
H, HKV, D = 32, 8, 64
NCORES = 8
ROWS = B * S              # 4096
RPC = ROWS // NCORES      # 512 output rows per core
HQ = H // NCORES          # 4 q heads per core
QCOLS = HQ * D            # 256
NCH = ROWS // 512         # 8 row chunks
NKT = E // 128            # 16 k-tiles over E
SKT = S // 128            # 16 key tiles per batch

_CACHE = {}
_VONES = np.ones((128, B * (S // 128) * 65), dtype=np.float32)


def _build_module():
    from contextlib import ExitStack

    import concourse.tile as tile
    from concourse import bacc, mybir

    dt = mybir.dt
    f32, f32r, bf16 = dt.float32, dt.float32r, dt.bfloat16
    EXP = mybir.ActivationFunctionType.Exp
    MULT = mybir.AluOpType.mult
    ADD = mybir.AluOpType.add

    nc = bacc.Bacc("TRN2", target_bir_lowering=False, debug=False, num_devices=NCORES)

    xT = nc.dram_tensor("xT", [E, ROWS], f32r, kind="ExternalInput")
    wq = nc.dram_tensor("wq", [128, NKT * QCOLS], f32r, kind="ExternalInput")
    wkv = nc.dram_tensor("wkv", [128, NKT * 128], f32r, kind="ExternalInput")
    cosE = nc.dram_tensor("cosE", [128, S], f32, kind="ExternalInput")
    sinE = nc.dram_tensor("sinE", [128, S], f32, kind="ExternalInput")
    perm = nc.dram_tensor("perm", [128, 128], f32r, kind="ExternalInput")
    ident = nc.dram_tensor("ident", [128, 64], f32, kind="ExternalInput")
    outw = nc.dram_tensor("outw", [E, E], f32r, kind="ExternalInput")
    biasr = nc.dram_tensor("biasr", [128, E], f32, kind="ExternalInput")
    vones = nc.dram_tensor("vones", [128, B * SKT * 65], f32r, kind="ExternalInput")
    out = nc.dram_tensor("out", [RPC, E], f32, kind="ExternalOutput")

    with tile.TileContext(nc) as tc, ExitStack() as ctx:
        persist = ctx.enter_context(tc.tile_pool(name="persist", bufs=1))
        dram = ctx.enter_context(tc.tile_pool(name="dram", bufs=1, space="DRAM"))

        qT0 = persist.tile([128, ROWS], f32r, tag="qT0")  # heads 0,1 (local), D-major
        qT1 = persist.tile([128, ROWS], f32r, tag="qT1")  # heads 2,3
        kT2 = persist.tile([128, ROWS], f32r, tag="kT2")  # roped kT duplicated on 0:64 / 64:128
        vsb = persist.tile([128, B * SKT * 65], f32r, tag="vsb")  # rows-major v + ones col
        perm_sb = persist.tile([128, 128], f32r, tag="perm")
        ident_sb = persist.tile([128, 64], f32, tag="ident")

        nc.sync.dma_start(perm_sb[:], perm[:])
        nc.sync.dma_start(ident_sb[:], ident[:])
        # ones columns of the augmented-V tile (data columns overwritten later)
        nc.sync.dma_start(vsb[:], vones[:])

        a2aA_in = dram.tile([NCORES, 128, RPC], f32r, tag="a2aA_in")
        a2aA_out = dram.tile([NCORES, 128, RPC], f32r, tag="a2aA_out")
        a2aB_in = dram.tile([NCORES, 128, RPC], f32r, tag="a2aB_in")
        a2aB_out = dram.tile([NCORES, 128, RPC], f32r, tag="a2aB_out")

        # ---------------- Phase A: QKV projections + RoPE + V transpose -------------
        with ExitStack() as ctxA, nc.named_scope("phaseA"):
            wpool = ctxA.enter_context(tc.tile_pool(name="wpool", bufs=1))
            xpool = ctxA.enter_context(tc.tile_pool(name="xpool", bufs=20))
            cspool = ctxA.enter_context(tc.tile_pool(name="cspool", bufs=2))
            tmpA = ctxA.enter_context(tc.tile_pool(name="tmpA", bufs=2))
            psA1 = ctxA.enter_context(tc.tile_pool(name="psA1", bufs=2, space="PSUM"))
            psA2 = ctxA.enter_context(tc.tile_pool(name="psA2", bufs=1, space="PSUM"))

            wq_sb = wpool.tile([128, NKT * QCOLS], f32r, tag="wq")
            wkv_sb = wpool.tile([128, NKT * 128], f32r, tag="wkv")
            nc.sync.dma_start(wq_sb[:], wq[:])
            nc.sync.dma_start(wkv_sb[:], wkv[:])

            for chp in range(NCH // 2):
                ps1k = slice(chp * 1024, (chp + 1) * 1024)
                xts = []
                for kt in range(NKT):
                    xt = xpool.tile([128, 1024], f32r, tag="xt")
                    nc.sync.dma_start(xt[:], xT[kt * 128 : (kt + 1) * 128, ps1k])
                    xts.append(xt)
                for sub in range(2):
                    ch = chp * 2 + sub
                    cs = slice(ch * 512, (ch + 1) * 512)
                    ss = slice(sub * 512, (sub + 1) * 512)
                    q0_ps = psA1.tile([128, 512], f32, tag="q0")
                    q1_ps = psA1.tile([128, 512], f32, tag="q1")
                    kv_ps = psA1.tile([128, 512], f32, tag="kv")
                    for kt in range(NKT):
                        st, sp = kt == 0, kt == NKT - 1
                        xs = xts[kt][:, ss]
                        wqk = wq_sb[:, kt * QCOLS : kt * QCOLS + 128]
                        wqk2 = wq_sb[:, kt * QCOLS + 128 : kt * QCOLS + 256]
                        nc.tensor.matmul(q0_ps[:], wqk, xs, start=st, stop=sp)
                        nc.tensor.matmul(q1_ps[:], wqk2, xs, start=st, stop=sp)
                        nc.tensor.matmul(
                            kv_ps[:], wkv_sb[:, kt * 128 : (kt + 1) * 128], xs,
                            start=st, stop=sp,
                        )

                    # PSUM -> SBUF (rounds to f32r)
                    nc.scalar.copy(qT0[:, cs], q0_ps[:])
                    nc.scalar.copy(qT1[:, cs], q1_ps[:])
                    nc.scalar.copy(kT2[0:64, cs], kv_ps[0:64, :])
                    vtt = tmpA.tile([128, 512], f32, tag="vtt")
                    nc.scalar.copy(vtt[64:128, :], kv_ps[64:128, :])

                    # RoPE: t = t*cosE + (perm @ t)*sinE   (in place)
                    scs = slice((ch % 4) * 512, (ch % 4 + 1) * 512)  # pos = row % S
                    cos_sb = cspool.tile([128, 512], f32, tag="cos")
                    sin_sb = cspool.tile([128, 512], f32, tag="sin")
                    nc.sync.dma_start(cos_sb[:], cosE[:, scs])
                    nc.sync.dma_start(sin_sb[:], sinE[:, scs])
                    for t, p in ((qT0, 128), (qT1, 128), (kT2, 64)):
                        rot_ps = psA2.tile([128, 512], f32, tag="rot")
                        nc.tensor.matmul(
                            rot_ps[0:p, :], perm_sb[0:p, 0:p], t[0:p, cs],
                            start=True, stop=True,
                        )
                        tmp = tmpA.tile([128, 512], f32, tag="ropetmp")
                        nc.vector.scalar_tensor_tensor(
                            out=tmp[0:p, :], in0=rot_ps[0:p, :], scalar=1.0,
                            in1=sin_sb[0:p, :], op0=MULT, op1=MULT,
                        )
                        nc.vector.scalar_tensor_tensor(
                            out=t[0:p, cs], in0=t[0:p, cs], scalar=1.0,
                            in1=cos_sb[0:p, :], op0=MULT, op1=MULT,
                        )
                        nc.vector.scalar_tensor_tensor(
                            out=t[0:p, cs], in0=t[0:p, cs], scalar=1.0,
                            in1=tmp[0:p, :], op0=MULT, op1=ADD,
                        )
                    # duplicate roped k on partitions 64:128 (for row-group packing)
                    nc.sync.dma_start(kT2[64:128, cs], kT2[0:64, cs])

                    # V transpose: [64,512] (keys on free) -> 4x [128,64] rows-major
                    b = ch // 4
                    for j in range(4):
                        kt_key = (ch % 4) * 4 + j
                        v_ps = psA2.tile([128, 64], f32, tag="vps")
                        nc.tensor.transpose(
                            v_ps[:], vtt[64:128, j * 128 : (j + 1) * 128],
                            ident_sb[64:128, :],
                        )
                        blk = (b * SKT + kt_key) * 65
                        nc.vector.tensor_copy(vsb[:, blk : blk + 64], v_ps[:])

        # ---------------- Phase C pools opened early so out_w prefetch overlaps B ---
        ctxC = ctx.enter_context(ExitStack())
        cpool = ctxC.enter_context(tc.tile_pool(name="cpool", bufs=1))
        wcolp = ctxC.enter_context(tc.tile_pool(name="wcolp", bufs=24))
        obuf = ctxC.enter_context(tc.tile_pool(name="obuf", bufs=3))
        oTown = cpool.tile([128, NKT * RPC], f32r, tag="oTown")
        bias_sb = cpool.tile([128, E], f32, tag="bias")
        nc.sync.dma_start(bias_sb[:], biasr[:])
        wcols0 = []
        for kt in range(NKT):
            wc = wcolp.tile([128, 512], f32r, tag="wc")
            nc.sync.dma_start(wc[:], outw[kt * 128 : (kt + 1) * 128, 0:512])
            wcols0.append(wc)
        wcols1 = []
        for kt in range(8):
            wc = wcolp.tile([128, 512], f32r, tag="wc")
            nc.sync.dma_start(wc[:], outw[kt * 128 : (kt + 1) * 128, 512:1024])
            wcols1.append(wc)

        # ---------------- Phase B: attention (scoresT -> exp -> A@V) ----------------
        with ExitStack() as ctxB, nc.named_scope("phaseB"):
            expool = ctxB.enter_context(tc.tile_pool(name="expool", bufs=3))
            rpool = ctxB.enter_context(tc.tile_pool(name="rpool", bufs=2))
            rdram = ctxB.enter_context(tc.tile_pool(name="rdram", bufs=2, space="DRAM"))
            onorm = ctxB.enter_context(tc.tile_pool(name="onorm", bufs=3))
            psB = ctxB.enter_context(tc.tile_pool(name="psB", bufs=2, space="PSUM"))
            psO = ctxB.enter_context(tc.tile_pool(name="psO", bufs=2, space="PSUM"))

            for hp, qTt in ((0, qT0), (1, qT1)):
                a2a_buf = a2aA_in if hp == 0 else a2aB_in
                for j in range(NCORES):  # output row block = a2a destination core
                    b, qc = j // 4, j % 4
                    qs = slice(b * S + qc * 512, b * S + (qc + 1) * 512)
                    oT_ps = psO.tile([65, 1024], f32, tag="oT")
                    for kt in range(SKT):
                        ks = slice(b * S + kt * 128, b * S + (kt + 1) * 128)
                        sc = psB.tile([128, 1024], f32, tag="sc")
                        nc.tensor.matmul(
                            sc[:, 0:512], kT2[0:64, ks], qTt[0:64, qs],
                            start=True, stop=True,
                        )
                        nc.tensor.matmul(
                            sc[:, 512:1024], kT2[64:128, ks], qTt[64:128, qs],
                            start=True, stop=True,
                        )
                        ex = expool.tile([128, 1024], f32r, tag="ex")
                        nc.scalar.activation(ex[:], sc[:], EXP, scale=0.125)
                        blk = (b * SKT + kt) * 65
                        st, sp = kt == 0, kt == SKT - 1
                        nc.tensor.matmul(
                            oT_ps[:, 0:512], vsb[:, blk : blk + 65], ex[:, 0:512],
                            start=st, stop=sp,
                        )
                        nc.tensor.matmul(
                            oT_ps[:, 512:1024], vsb[:, blk : blk + 65],
                            ex[:, 512:1024], start=st, stop=sp,
                        )
                    # normalize by the ones-row sum; write straight into a2a buffer
                    for hh in range(2):
                        hs = slice(hh * 512, (hh + 1) * 512)
                        rc = rpool.tile([1, 512], f32, tag="rc")
                        nc.vector.reciprocal(out=rc[:], in_=oT_ps[64:65, hs])
                        rcd = rdram.tile([1, 512], f32, tag="rcd")
                        nc.sync.dma_start(rcd[:], rc[:])
                        rb = rpool.tile([64, 512], f32, tag="rb")
                        nc.sync.dma_start(rb[:], rcd[0:1, :].to_broadcast((64, 512)))
                        on = onorm.tile([64, 512], f32r, tag="on")
                        nc.vector.scalar_tensor_tensor(
                            out=on[:], in0=oT_ps[0:64, hs], scalar=1.0,
                            in1=rb[:], op0=MULT, op1=MULT,
                        )
                        nc.sync.dma_start(
                            a2a_buf[j, hh * 64 : (hh + 1) * 64, :], on[:]
                        )
                with nc.named_scope(f"a2a{hp}"):
                    nc.gpsimd.collective_compute(
                        "AllToAll",
                        mybir.AluOpType.bypass,
                        replica_groups=[list(range(NCORES))],
                        ins=[(a2aA_in if hp == 0 else a2aB_in).opt()],
                        outs=[(a2aA_out if hp == 0 else a2aB_out).opt()],
                    )

        # ---------------- Phase C: output projection for own row slice --------------
        with nc.named_scope("phaseC"):
            psC = ctxC.enter_context(tc.tile_pool(name="psC", bufs=2, space="PSUM"))

            for kt in range(NKT):
                src_t = a2aA_out if kt % 2 == 0 else a2aB_out
                nc.sync.dma_start(
                    oTown[:, kt * RPC : (kt + 1) * RPC], src_t[kt // 2, :, :]
                )

            for nch in range(4):
                ns = slice(nch * 512, (nch + 1) * 512)
                if nch == 0:
                    wcols = wcols0
                else:
                    wcols = wcols1 if nch == 1 else []
                    for kt in range(len(wcols), NKT):
                        wc = wcolp.tile([128, 512], f32r, tag="wc")
                        nc.sync.dma_start(wc[:], outw[kt * 128 : (kt + 1) * 128, ns])
                        wcols.append(wc)
                for mt in range(4):
                    acc = psC.tile([128, 512], f32, tag="acc")
                    kt_order = list(range(0, NKT, 2)) + list(range(1, NKT, 2))
                    for i, kt in enumerate(kt_order):
                        nc.tensor.matmul(
                            acc[:],
                            oTown[:, kt * RPC + mt * 128 : kt * RPC + (mt + 1) * 128],
                            wcols[kt][:],
                            start=(i == 0), stop=(i == NKT - 1),
                        )
                    ob = obuf.tile([128, 512], f32, tag="ob")
                    nc.vector.scalar_tensor_tensor(
                        out=ob[:], in0=acc[:], scalar=1.0,
                        in1=bias_sb[:, ns], op0=MULT, op1=ADD,
                    )
                    nc.sync.dma_start(out[mt * 128 : (mt + 1) * 128, ns], ob[:])

    nc.finalize()
    return nc


def _prep_inputs(x, freqs_cos, freqs_sin, wq, wk, wv, out_w, out_b):
    x2 = np.ascontiguousarray(np.asarray(x, dtype=np.float32).reshape(ROWS, E))
    xT = np.ascontiguousarray(x2.T)

    cos = np.asarray(freqs_cos, dtype=np.float32).reshape(S, D // 2)
    sin = np.asarray(freqs_sin, dtype=np.float32).reshape(S, D // 2)
    cos_exp = np.repeat(cos.T, 2, axis=0)            # [64, S]
    sin_exp = np.repeat(sin.T, 2, axis=0)
    sin_exp[0::2] *= -1.0                            # -sin on even rows
    cosE = np.ascontiguousarray(np.tile(cos_exp, (2, 1)))  # [128, S]
    sinE = np.ascontiguousarray(np.tile(sin_exp, (2, 1)))

    perm = np.zeros((128, 128), dtype=np.float32)
    idx = np.arange(64)
    perm[2 * idx, 2 * idx + 1] = 1.0
    perm[2 * idx + 1, 2 * idx] = 1.0

    ident = np.tile(np.eye(64, dtype=np.float32), (2, 1))  # [128, 64]

    wq_f = np.asarray(wq, dtype=np.float32)
    wk_f = np.asarray(wk, dtype=np.float32)
    wv_f = np.asarray(wv, dtype=np.float32)
    outw_f = np.ascontiguousarray(np.asarray(out_w, dtype=np.float32))
    biasr = np.ascontiguousarray(
        np.tile(np.asarray(out_b, dtype=np.float32)[None, :], (128, 1))
    )

    in_maps = []
    for c in range(NCORES):
        wq_c = np.ascontiguousarray(
            wq_f[:, c * QCOLS : (c + 1) * QCOLS]
            .reshape(NKT, 128, QCOLS).transpose(1, 0, 2).reshape(128, NKT * QCOLS)
        )
        wkv_c = np.ascontiguousarray(
            np.concatenate(
                [wk_f[:, c * 64 : (c + 1) * 64], wv_f[:, c * 64 : (c + 1) * 64]],
                axis=1,
            ).reshape(NKT, 128, 128).transpose(1, 0, 2).reshape(128, NKT * 128)
        )
        in_maps.append(
            {
                "xT": xT, "wq": wq_c, "wkv": wkv_c, "cosE": cosE, "sinE": sinE,
                "perm": perm, "ident": ident, "outw": outw_f, "biasr": biasr,
                "vones": _VONES,
            }
        )
    return in_maps


def kernel(
    x, start_pos, freqs_cos, freqs_sin, wq, wk, wv, out_w, out_b,
    k_cache=None, v_cache=None, _trace=False, _trace_cores=None,
):
    from concourse.bass_utils import run_bass_kernel_spmd

    sp = int(np.asarray(start_pos))
    assert sp == 0, f"kernel specialized for start_pos=0, got {sp}"

    if "nc" not in _CACHE:
        _CACHE["nc"] = _build_module()
    nc = _CACHE["nc"]

    in_maps = _prep_inputs(x, freqs_cos, freqs_sin, wq, wk, wv, out_w, out_b)

    kwargs = {}
    if _trace:
        _install_ntff_hook()
        kwargs = {"trace": True, "trace_cores": _trace_cores}
    res = run_bass_kernel_spmd(nc, in_maps, list(range(NCORES)), **kwargs)

    full = np.concatenate([res.results[c]["out"] for c in range(NCORES)], axis=0)
    out = full.reshape(B, S, E).astype(np.float32)
    if _trace:
        return out, res
    return out


def _install_ntff_hook():
    """The agent image lacks antenv.axon_hooks; synthesize it so trace=True works."""
    import sys, types

    if "antenv.axon_hooks" in sys.modules:
        return
    try:
        from trn_agent_boot.trn_boot import _ntff_profile_via_ctypes

        hook = _ntff_profile_via_ctypes("/opt/axon/libaxon_pjrt.so")
    except Exception:
        hook = None
    mod = types.ModuleType("antenv.axon_hooks")
    mod.get_axon_ntff_profile_hook = lambda: hook
    sys.modules["antenv.axon_hooks"] = mod


# revision 23
# speedup vs baseline: 1.0535x; 1.0535x over previous
"""GQA attention block (B=2, S=2048, E=2048, H=32, HKV=8, D=64) on 8 trn2 cores.

Sharding: tensor-parallel over heads. Core c owns q-heads 4c..4c+3 and kv-head c.
Each core computes its heads' attention for ALL rows, then an AllToAll exchanges
head-blocks for row-blocks so each core runs the output projection for its own
512-row slice against the full out_w. Host concatenates row slices.

All matmuls run as float32r (TF32-like, full PE rate at N>=512). Softmax is
computed without max-subtraction (scores are O(4), exp cannot overflow), with
denominators obtained by augmenting V with a ones column.
"""

import numpy as np

B, S, E = 2, 2048, 2048
H, HKV, D = 32, 8, 64
NCORES = 8
ROWS = B * S              # 4096
RPC = ROWS // NCORES      # 512 output rows per core
HQ = H // NCORES          # 4 q heads per core
QCOLS = HQ * D            # 256
NCH = ROWS // 512         # 8 row chunks
NKT = E // 128            # 16 k-tiles over E
SKT = S // 128            # 16 key tiles per batch

_CACHE = {}
_VONES = np.ones((128, B * (S // 128) * 65), dtype=np.float32)


def _build_module():
    from contextlib import ExitStack

    import concourse.tile as tile
    from concourse import bacc, mybir

    dt = mybir.dt
    f32, f32r, bf16 = dt.float32, dt.float32r, dt.bfloat16
    EXP = mybir.ActivationFunctionType.Exp
    MULT = mybir.AluOpType.mult
    ADD = mybir.AluOpType.add

    nc = bacc.Bacc("TRN2", target_bir_lowering=False, debug=False, num_devices=NCORES)

    xT = nc.dram_tensor("xT", [E, ROWS], f32r, kind="ExternalInput")
    wq = nc.dram_tensor("wq", [128, NKT * QCOLS], f32r, kind="ExternalInput")
    wkv = nc.dram_tensor("wkv", [128, NKT * 128], f32r, kind="ExternalInput")
    cosE = nc.dram_tensor("cosE", [128, S], f32, kind="ExternalInput")
    sinE = nc.dram_tensor("sinE", [128, S], f32, kind="ExternalInput")
    perm = nc.dram_tensor("perm", [128, 128], f32r, kind="ExternalInput")
    ident = nc.dram_tensor("ident", [128, 64], f32, kind="ExternalInput")
    outw = nc.dram_tensor("outw", [E, E], f32r, kind="ExternalInput")
    biasr = nc.dram_tensor("biasr", [128, E], f32, kind="ExternalInput")
    vones = nc.dram_tensor("vones", [128, B * SKT * 65], f32r, kind="ExternalInput")
    out = nc.dram_tensor("out", [RPC, E], f32, kind="ExternalOutput")

    with tile.TileContext(nc) as tc, ExitStack() as ctx:
        persist = ctx.enter_context(tc.tile_pool(name="persist", bufs=1))
        dram = ctx.enter_context(tc.tile_pool(name="dram", bufs=1, space="DRAM"))

        qT0 = persist.tile([128, ROWS], f32r, tag="qT0")  # heads 0,1 (local), D-major
        qT1 = persist.tile([128, ROWS], f32r, tag="qT1")  # heads 2,3
        kT2 = persist.tile([128, ROWS], f32r, tag="kT2")  # roped kT duplicated on 0:64 / 64:128
        vsb = persist.tile([128, B * SKT * 65], f32r, tag="vsb")  # rows-major v + ones col
        perm_sb = persist.tile([128, 128], f32r, tag="perm")
        ident_sb = persist.tile([128, 64], f32, tag="ident")

        nc.sync.dma_start(perm_sb[:], perm[:])
        nc.sync.dma_start(ident_sb[:], ident[:])
        # ones columns of the augmented-V tile (data columns overwritten later)
        nc.sync.dma_start(vsb[:], vones[:])

        a2aA_in = dram.tile([NCORES, 128, RPC], f32r, tag="a2aA_in")
        a2aA_out = dram.tile([NCORES, 128, RPC], f32r, tag="a2aA_out")
        a2aB_in = dram.tile([NCORES, 128, RPC], f32r, tag="a2aB_in")
        a2aB_out = dram.tile([NCORES, 128, RPC], f32r, tag="a2aB_out")

        # ---------------- Phase A: QKV projections + RoPE + V transpose -------------
        with ExitStack() as ctxA, nc.named_scope("phaseA"):
            wpool = ctxA.enter_context(tc.tile_pool(name="wpool", bufs=1))
            xpool = ctxA.enter_context(tc.tile_pool(name="xpool", bufs=20))
            cspool = ctxA.enter_context(tc.tile_pool(name="cspool", bufs=2))
            tmpA = ctxA.enter_context(tc.tile_pool(name="tmpA", bufs=2))
            psA1 = ctxA.enter_context(tc.tile_pool(name="psA1", bufs=2, space="PSUM"))
            psA2 = ctxA.enter_context(tc.tile_pool(name="psA2", bufs=1, space="PSUM"))

            wq_sb = wpool.tile([128, NKT * QCOLS], f32r, tag="wq")
            wkv_sb = wpool.tile([128, NKT * 128], f32r, tag="wkv")
            nc.sync.dma_start(wq_sb[:], wq[:])
            nc.sync.dma_start(wkv_sb[:], wkv[:])

            for chp in range(NCH // 2):
                ps1k = slice(chp * 1024, (chp + 1) * 1024)
                xts = []
                for kt in range(NKT):
                    xt = xpool.tile([128, 1024], f32r, tag="xt")
                    nc.sync.dma_start(xt[:], xT[kt * 128 : (kt + 1) * 128, ps1k])
                    xts.append(xt)
                for sub in range(2):
                    ch = chp * 2 + sub
                    cs = slice(ch * 512, (ch + 1) * 512)
                    ss = slice(sub * 512, (sub + 1) * 512)
                    q0_ps = psA1.tile([128, 512], f32, tag="q0")
                    q1_ps = psA1.tile([128, 512], f32, tag="q1")
                    kv_ps = psA1.tile([128, 512], f32, tag="kv")
                    for kt in range(NKT):
                        st, sp = kt == 0, kt == NKT - 1
                        xs = xts[kt][:, ss]
                        wqk = wq_sb[:, kt * QCOLS : kt * QCOLS + 128]
                        wqk2 = wq_sb[:, kt * QCOLS + 128 : kt * QCOLS + 256]
                        nc.tensor.matmul(q0_ps[:], wqk, xs, start=st, stop=sp)
                        nc.tensor.matmul(q1_ps[:], wqk2, xs, start=st, stop=sp)
                        nc.tensor.matmul(
                            kv_ps[:], wkv_sb[:, kt * 128 : (kt + 1) * 128], xs,
                            start=st, stop=sp,
                        )

                    # PSUM -> SBUF (rounds to f32r)
                    nc.scalar.copy(qT0[:, cs], q0_ps[:])
                    nc.scalar.copy(qT1[:, cs], q1_ps[:])
                    nc.scalar.copy(kT2[0:64, cs], kv_ps[0:64, :])
                    vtt = tmpA.tile([128, 512], f32, tag="vtt")
                    nc.scalar.copy(vtt[64:128, :], kv_ps[64:128, :])

                    # RoPE: t = t*cosE + (perm @ t)*sinE   (in place)
                    scs = slice((ch % 4) * 512, (ch % 4 + 1) * 512)  # pos = row % S
                    cos_sb = cspool.tile([128, 512], f32, tag="cos")
                    sin_sb = cspool.tile([128, 512], f32, tag="sin")
                    nc.sync.dma_start(cos_sb[:], cosE[:, scs])
                    nc.sync.dma_start(sin_sb[:], sinE[:, scs])
                    for t, p in ((qT0, 128), (qT1, 128), (kT2, 64)):
                        rot_ps = psA2.tile([128, 512], f32, tag="rot")
                        nc.tensor.matmul(
                            rot_ps[0:p, :], perm_sb[0:p, 0:p], t[0:p, cs],
                            start=True, stop=True,
                        )
                        tmp = tmpA.tile([128, 512], f32, tag="ropetmp")
                        nc.vector.scalar_tensor_tensor(
                            out=tmp[0:p, :], in0=rot_ps[0:p, :], scalar=1.0,
                            in1=sin_sb[0:p, :], op0=MULT, op1=MULT,
                        )
                        nc.vector.scalar_tensor_tensor(
                            out=t[0:p, cs], in0=t[0:p, cs], scalar=1.0,
                            in1=cos_sb[0:p, :], op0=MULT, op1=MULT,
                        )
                        nc.vector.scalar_tensor_tensor(
                            out=t[0:p, cs], in0=t[0:p, cs], scalar=1.0,
                            in1=tmp[0:p, :], op0=MULT, op1=ADD,
                        )
                    # duplicate roped k on partitions 64:128 (for row-group packing)
                    nc.sync.dma_start(kT2[64:128, cs], kT2[0:64, cs])

                    # V transpose: [64,512] (keys on free) -> 4x [128,64] rows-major
                    b = ch // 4
                    for j in range(4):
                        kt_key = (ch % 4) * 4 + j
                        v_ps = psA2.tile([128, 64], f32, tag="vps")
                        nc.tensor.transpose(
                            v_ps[:], vtt[64:128, j * 128 : (j + 1) * 128],
                            ident_sb[64:128, :],
                        )
                        blk = (b * SKT + kt_key) * 65
                        nc.vector.tensor_copy(vsb[:, blk : blk + 64], v_ps[:])

        # ---------------- Phase C pools opened early so out_w prefetch overlaps B ---
        ctxC = ctx.enter_context(ExitStack())
        cpool = ctxC.enter_context(tc.tile_pool(name="cpool", bufs=1))
        wcolp = ctxC.enter_context(tc.tile_pool(name="wcolp", bufs=24))
        obuf = ctxC.enter_context(tc.tile_pool(name="obuf", bufs=4))
        oTown = cpool.tile([128, NKT * RPC], f32r, tag="oTown")
        bias_sb = cpool.tile([128, E], f32, tag="bias")
        nc.sync.dma_start(bias_sb[:], biasr[:])
        wcols0 = []
        for kt in range(NKT):
            wc = wcolp.tile([128, 512], f32r, tag="wc")
            nc.sync.dma_start(wc[:], outw[kt * 128 : (kt + 1) * 128, 0:512])
            wcols0.append(wc)
        wcols1 = []
        for kt in range(8):
            wc = wcolp.tile([128, 512], f32r, tag="wc")
            nc.sync.dma_start(wc[:], outw[kt * 128 : (kt + 1) * 128, 512:1024])
            wcols1.append(wc)

        # ---------------- Phase B: attention (scoresT -> exp -> A@V) ----------------
        with ExitStack() as ctxB, nc.named_scope("phaseB"):
            expool = ctxB.enter_context(tc.tile_pool(name="expool", bufs=3))
            rpool = ctxB.enter_context(tc.tile_pool(name="rpool", bufs=3))
            rdram = ctxB.enter_context(tc.tile_pool(name="rdram", bufs=3, space="DRAM"))
            onorm = ctxB.enter_context(tc.tile_pool(name="onorm", bufs=4))
            psB = ctxB.enter_context(tc.tile_pool(name="psB", bufs=2, space="PSUM"))
            psO = ctxB.enter_context(tc.tile_pool(name="psO", bufs=2, space="PSUM"))

            for hp, qTt in ((0, qT0), (1, qT1)):
                a2a_buf = a2aA_in if hp == 0 else a2aB_in
                for j in range(NCORES):  # output row block = a2a destination core
                    b, qc = j // 4, j % 4
                    qs = slice(b * S + qc * 512, b * S + (qc + 1) * 512)
                    oT_ps = psO.tile([65, 1024], f32, tag="oT")
                    for kt in range(SKT):
                        ks = slice(b * S + kt * 128, b * S + (kt + 1) * 128)
                        sc = psB.tile([128, 1024], f32, tag="sc")
                        nc.tensor.matmul(
                            sc[:, 0:512], kT2[0:64, ks], qTt[0:64, qs],
                            start=True, stop=True,
                        )
                        nc.tensor.matmul(
                            sc[:, 512:1024], kT2[64:128, ks], qTt[64:128, qs],
                            start=True, stop=True,
                        )
                        ex = expool.tile([128, 1024], f32r, tag="ex")
                        nc.scalar.activation(ex[:], sc[:], EXP, scale=0.125)
                        blk = (b * SKT + kt) * 65
                        st, sp = kt == 0, kt == SKT - 1
                        nc.tensor.matmul(
                            oT_ps[:, 0:512], vsb[:, blk : blk + 65], ex[:, 0:512],
                            start=st, stop=sp,
                        )
                        nc.tensor.matmul(
                            oT_ps[:, 512:1024], vsb[:, blk : blk + 65],
                            ex[:, 512:1024], start=st, stop=sp,
                        )
                    # normalize by the ones-row sum; write straight into a2a buffer
                    for hh in range(2):
                        hs = slice(hh * 512, (hh + 1) * 512)
                        rc = rpool.tile([1, 512], f32, tag="rc")
                        nc.vector.reciprocal(out=rc[:], in_=oT_ps[64:65, hs])
                        rcd = rdram.tile([1, 512], f32, tag="rcd")
                        nc.sync.dma_start(rcd[:], rc[:])
                        rb = rpool.tile([64, 512], f32, tag="rb")
                        nc.sync.dma_start(rb[:], rcd[0:1, :].to_broadcast((64, 512)))
                        on = onorm.tile([64, 512], f32r, tag="on")
                        nc.vector.scalar_tensor_tensor(
                            out=on[:], in0=oT_ps[0:64, hs], scalar=1.0,
                            in1=rb[:], op0=MULT, op1=MULT,
                        )
                        nc.sync.dma_start(
                            a2a_buf[j, hh * 64 : (hh + 1) * 64, :], on[:]
                        )
                with nc.named_scope(f"a2a{hp}"):
                    nc.gpsimd.collective_compute(
                        "AllToAll",
                        mybir.AluOpType.bypass,
                        replica_groups=[list(range(NCORES))],
                        ins=[(a2aA_in if hp == 0 else a2aB_in).opt()],
                        outs=[(a2aA_out if hp == 0 else a2aB_out).opt()],
                    )

        # ---------------- Phase C: output projection for own row slice --------------
        with nc.named_scope("phaseC"):
            psC = ctxC.enter_context(tc.tile_pool(name="psC", bufs=3, space="PSUM"))

            for kt in range(NKT):
                src_t = a2aA_out if kt % 2 == 0 else a2aB_out
                nc.sync.dma_start(
                    oTown[:, kt * RPC : (kt + 1) * RPC], src_t[kt // 2, :, :]
                )

            for nch in range(4):
                ns = slice(nch * 512, (nch + 1) * 512)
                if nch == 0:
                    wcols = wcols0
                else:
                    wcols = wcols1 if nch == 1 else []
                    for kt in range(len(wcols), NKT):
                        wc = wcolp.tile([128, 512], f32r, tag="wc")
                        nc.sync.dma_start(wc[:], outw[kt * 128 : (kt + 1) * 128, ns])
                        wcols.append(wc)
                for mt in range(4):
                    acc = psC.tile([128, 512], f32, tag="acc")
                    kt_order = list(range(0, NKT, 2)) + list(range(1, NKT, 2))
                    for i, kt in enumerate(kt_order):
                        nc.tensor.matmul(
                            acc[:],
                            oTown[:, kt * RPC + mt * 128 : kt * RPC + (mt + 1) * 128],
                            wcols[kt][:],
                            start=(i == 0), stop=(i == NKT - 1),
                        )
                    ob = obuf.tile([128, 512], f32, tag="ob")
                    nc.vector.scalar_tensor_tensor(
                        out=ob[:], in0=acc[:], scalar=1.0,
                        in1=bias_sb[:, ns], op0=MULT, op1=ADD,
                    )
                    nc.sync.dma_start(out[mt * 128 : (mt + 1) * 128, ns], ob[:])

    nc.finalize()
    return nc


def _prep_inputs(x, freqs_cos, freqs_sin, wq, wk, wv, out_w, out_b):
    x2 = np.ascontiguousarray(np.asarray(x, dtype=np.float32).reshape(ROWS, E))
    xT = np.ascontiguousarray(x2.T)

    cos = np.asarray(freqs_cos, dtype=np.float32).reshape(S, D // 2)
    sin = np.asarray(freqs_sin, dtype=np.float32).reshape(S, D // 2)
    cos_exp = np.repeat(cos.T, 2, axis=0)            # [64, S]
    sin_exp = np.repeat(sin.T, 2, axis=0)
    sin_exp[0::2] *= -1.0                            # -sin on even rows
    cosE = np.ascontiguousarray(np.tile(cos_exp, (2, 1)))  # [128, S]
    sinE = np.ascontiguousarray(np.tile(sin_exp, (2, 1)))

    perm = np.zeros((128, 128), dtype=np.float32)
    idx = np.arange(64)
    perm[2 * idx, 2 * idx + 1] = 1.0
    perm[2 * idx + 1, 2 * idx] = 1.0

    ident = np.tile(np.eye(64, dtype=np.float32), (2, 1))  # [128, 64]

    wq_f = np.asarray(wq, dtype=np.float32)
    wk_f = np.asarray(wk, dtype=np.float32)
    wv_f = np.asarray(wv, dtype=np.float32)
    outw_f = np.ascontiguousarray(np.asarray(out_w, dtype=np.float32))
    biasr = np.ascontiguousarray(
        np.tile(np.asarray(out_b, dtype=np.float32)[None, :], (128, 1))
    )

    in_maps = []
    for c in range(NCORES):
        wq_c = np.ascontiguousarray(
            wq_f[:, c * QCOLS : (c + 1) * QCOLS]
            .reshape(NKT, 128, QCOLS).transpose(1, 0, 2).reshape(128, NKT * QCOLS)
        )
        wkv_c = np.ascontiguousarray(
            np.concatenate(
                [wk_f[:, c * 64 : (c + 1) * 64], wv_f[:, c * 64 : (c + 1) * 64]],
                axis=1,
            ).reshape(NKT, 128, 128).transpose(1, 0, 2).reshape(128, NKT * 128)
        )
        in_maps.append(
            {
                "xT": xT, "wq": wq_c, "wkv": wkv_c, "cosE": cosE, "sinE": sinE,
                "perm": perm, "ident": ident, "outw": outw_f, "biasr": biasr,
                "vones": _VONES,
            }
        )
    return in_maps


def kernel(
    x, start_pos, freqs_cos, freqs_sin, wq, wk, wv, out_w, out_b,
    k_cache=None, v_cache=None, _trace=False, _trace_cores=None,
):
    from concourse.bass_utils import run_bass_kernel_spmd

    sp = int(np.asarray(start_pos))
    assert sp == 0, f"kernel specialized for start_pos=0, got {sp}"

    if "nc" not in _CACHE:
        _CACHE["nc"] = _build_module()
    nc = _CACHE["nc"]

    in_maps = _prep_inputs(x, freqs_cos, freqs_sin, wq, wk, wv, out_w, out_b)

    kwargs = {}
    if _trace:
        _install_ntff_hook()
        kwargs = {"trace": True, "trace_cores": _trace_cores}
    res = run_bass_kernel_spmd(nc, in_maps, list(range(NCORES)), **kwargs)

    full = np.concatenate([res.results[c]["out"] for c in range(NCORES)], axis=0)
    out = full.reshape(B, S, E).astype(np.float32)
    if _trace:
        return out, res
    return out


def _install_ntff_hook():
    """The agent image lacks antenv.axon_hooks; synthesize it so trace=True works."""
    import sys, types

    if "antenv.axon_hooks" in sys.modules:
        return
    try:
        from trn_agent_boot.trn_boot import _ntff_profile_via_ctypes

        hook = _ntff_profile_via_ctypes("/opt/axon/libaxon_pjrt.so")
    except Exception:
        hook = None
    mod = types.ModuleType("antenv.axon_hooks")
    mod.get_axon_ntff_profile_hook = lambda: hook
    sys.modules["antenv.axon_hooks"] = mod


# revision 25
# speedup vs baseline: 1.2050x; 1.1438x over previous
"""GQA attention block (B=2, S=2048, E=2048, H=32, HKV=8, D=64) on 8 trn2 cores.

Sharding: tensor-parallel over heads. Core c owns q-heads 4c..4c+3 and kv-head c.
Each core computes its heads' attention for ALL rows, then an AllToAll exchanges
head-blocks for row-blocks so each core runs the output projection for its own
512-row slice against the full out_w. Host concatenates row slices.

All matmuls run as float32r (TF32-like, full PE rate at N>=512). Softmax is
computed without max-subtraction (scores are O(4), exp cannot overflow), with
denominators obtained by augmenting V with a ones column.
"""

import numpy as np

B, S, E = 2, 2048, 2048
H, HKV, D = 32, 8, 64
NCORES = 8
ROWS = B * S              # 4096
RPC = ROWS // NCORES      # 512 output rows per core
HQ = H // NCORES          # 4 q heads per core
QCOLS = HQ * D            # 256
NCH = ROWS // 512         # 8 row chunks
NKT = E // 128            # 16 k-tiles over E
SKT = S // 128            # 16 key tiles per batch

_CACHE = {}
_VONES = np.ones((128, B * (S // 128) * 65), dtype=np.float16)


def _build_module():
    from contextlib import ExitStack

    import concourse.tile as tile
    from concourse import bacc, mybir

    dt = mybir.dt
    f32, f32r, bf16 = dt.float32, dt.float32r, dt.bfloat16
    f16 = dt.float16
    EXP = mybir.ActivationFunctionType.Exp
    MULT = mybir.AluOpType.mult
    ADD = mybir.AluOpType.add

    nc = bacc.Bacc("TRN2", target_bir_lowering=False, debug=False, num_devices=NCORES)

    xT = nc.dram_tensor("xT", [E, ROWS], f16, kind="ExternalInput")
    wq = nc.dram_tensor("wq", [128, NKT * QCOLS], f16, kind="ExternalInput")
    wkv = nc.dram_tensor("wkv", [128, NKT * 128], f16, kind="ExternalInput")
    cosE = nc.dram_tensor("cosE", [128, S], f32, kind="ExternalInput")
    sinE = nc.dram_tensor("sinE", [128, S], f32, kind="ExternalInput")
    perm = nc.dram_tensor("perm", [128, 128], f16, kind="ExternalInput")
    ident = nc.dram_tensor("ident", [128, 64], f16, kind="ExternalInput")
    outw = nc.dram_tensor("outw", [E, E], f16, kind="ExternalInput")
    biasr = nc.dram_tensor("biasr", [128, E], f32, kind="ExternalInput")
    vones = nc.dram_tensor("vones", [128, B * SKT * 65], f16, kind="ExternalInput")
    out = nc.dram_tensor("out", [RPC, E], f32, kind="ExternalOutput")

    with tile.TileContext(nc) as tc, ExitStack() as ctx:
        persist = ctx.enter_context(tc.tile_pool(name="persist", bufs=1))
        dram = ctx.enter_context(tc.tile_pool(name="dram", bufs=1, space="DRAM"))

        qT0 = persist.tile([128, ROWS], f16, tag="qT0")  # heads 0,1 (local), D-major
        qT1 = persist.tile([128, ROWS], f16, tag="qT1")  # heads 2,3
        kT2 = persist.tile([128, ROWS], f16, tag="kT2")  # roped kT duplicated on 0:64 / 64:128
        vsb = persist.tile([128, B * SKT * 65], f16, tag="vsb")  # rows-major v + ones col
        perm_sb = persist.tile([128, 128], f16, tag="perm")
        ident_sb = persist.tile([128, 64], f16, tag="ident")

        nc.sync.dma_start(perm_sb[:], perm[:])
        nc.sync.dma_start(ident_sb[:], ident[:])
        # ones columns of the augmented-V tile (data columns overwritten later)
        nc.sync.dma_start(vsb[:], vones[:])

        a2aA_in = dram.tile([NCORES, 128, RPC], f16, tag="a2aA_in")
        a2aA_out = dram.tile([NCORES, 128, RPC], f16, tag="a2aA_out")
        a2aB_in = dram.tile([NCORES, 128, RPC], f16, tag="a2aB_in")
        a2aB_out = dram.tile([NCORES, 128, RPC], f16, tag="a2aB_out")

        # ---------------- Phase A: QKV projections + RoPE + V transpose -------------
        with ExitStack() as ctxA, nc.named_scope("phaseA"):
            wpool = ctxA.enter_context(tc.tile_pool(name="wpool", bufs=1))
            xpool = ctxA.enter_context(tc.tile_pool(name="xpool", bufs=20))
            cspool = ctxA.enter_context(tc.tile_pool(name="cspool", bufs=2))
            tmpA = ctxA.enter_context(tc.tile_pool(name="tmpA", bufs=2))
            psA1 = ctxA.enter_context(tc.tile_pool(name="psA1", bufs=2, space="PSUM"))
            psA2 = ctxA.enter_context(tc.tile_pool(name="psA2", bufs=1, space="PSUM"))

            wq_sb = wpool.tile([128, NKT * QCOLS], f16, tag="wq")
            wkv_sb = wpool.tile([128, NKT * 128], f16, tag="wkv")
            nc.sync.dma_start(wq_sb[:], wq[:])
            nc.sync.dma_start(wkv_sb[:], wkv[:])

            for chp in range(NCH // 2):
                ps1k = slice(chp * 1024, (chp + 1) * 1024)
                xts = []
                for kt in range(NKT):
                    xt = xpool.tile([128, 1024], f16, tag="xt")
                    nc.sync.dma_start(xt[:], xT[kt * 128 : (kt + 1) * 128, ps1k])
                    xts.append(xt)
                for sub in range(2):
                    ch = chp * 2 + sub
                    cs = slice(ch * 512, (ch + 1) * 512)
                    ss = slice(sub * 512, (sub + 1) * 512)
                    q0_ps = psA1.tile([128, 512], f32, tag="q0")
                    q1_ps = psA1.tile([128, 512], f32, tag="q1")
                    kv_ps = psA1.tile([128, 512], f32, tag="kv")
                    for kt in range(NKT):
                        st, sp = kt == 0, kt == NKT - 1
                        xs = xts[kt][:, ss]
                        wqk = wq_sb[:, kt * QCOLS : kt * QCOLS + 128]
                        wqk2 = wq_sb[:, kt * QCOLS + 128 : kt * QCOLS + 256]
                        nc.tensor.matmul(q0_ps[:], wqk, xs, start=st, stop=sp)
                        nc.tensor.matmul(q1_ps[:], wqk2, xs, start=st, stop=sp)
                        nc.tensor.matmul(
                            kv_ps[:], wkv_sb[:, kt * 128 : (kt + 1) * 128], xs,
                            start=st, stop=sp,
                        )

                    # PSUM -> SBUF (rounds to f32r)
                    nc.scalar.copy(qT0[:, cs], q0_ps[:])
                    nc.scalar.copy(qT1[:, cs], q1_ps[:])
                    nc.scalar.copy(kT2[0:64, cs], kv_ps[0:64, :])
                    vtt = tmpA.tile([128, 512], f16, tag="vtt")
                    nc.scalar.copy(vtt[64:128, :], kv_ps[64:128, :])

                    # RoPE: t = t*cosE + (perm @ t)*sinE   (in place)
                    scs = slice((ch % 4) * 512, (ch % 4 + 1) * 512)  # pos = row % S
                    cos_sb = cspool.tile([128, 512], f32, tag="cos")
                    sin_sb = cspool.tile([128, 512], f32, tag="sin")
                    nc.sync.dma_start(cos_sb[:], cosE[:, scs])
                    nc.sync.dma_start(sin_sb[:], sinE[:, scs])
                    for t, p in ((qT0, 128), (qT1, 128), (kT2, 64)):
                        rot_ps = psA2.tile([128, 512], f32, tag="rot")
                        nc.tensor.matmul(
                            rot_ps[0:p, :], perm_sb[0:p, 0:p], t[0:p, cs],
                            start=True, stop=True,
                        )
                        tmp = tmpA.tile([128, 512], f32, tag="ropetmp")
                        nc.vector.scalar_tensor_tensor(
                            out=tmp[0:p, :], in0=rot_ps[0:p, :], scalar=1.0,
                            in1=sin_sb[0:p, :], op0=MULT, op1=MULT,
                        )
                        nc.vector.scalar_tensor_tensor(
                            out=t[0:p, cs], in0=t[0:p, cs], scalar=1.0,
                            in1=cos_sb[0:p, :], op0=MULT, op1=MULT,
                        )
                        nc.vector.scalar_tensor_tensor(
                            out=t[0:p, cs], in0=t[0:p, cs], scalar=1.0,
                            in1=tmp[0:p, :], op0=MULT, op1=ADD,
                        )
                    # duplicate roped k on partitions 64:128 (for row-group packing)
                    nc.sync.dma_start(kT2[64:128, cs], kT2[0:64, cs])

                    # V transpose: [64,512] (keys on free) -> 4x [128,64] rows-major
                    b = ch // 4
                    for j in range(4):
                        kt_key = (ch % 4) * 4 + j
                        v_ps = psA2.tile([128, 64], f16, tag="vps")
                        nc.tensor.transpose(
                            v_ps[:], vtt[64:128, j * 128 : (j + 1) * 128],
                            ident_sb[64:128, :],
                        )
                        blk = (b * SKT + kt_key) * 65
                        nc.vector.tensor_copy(vsb[:, blk : blk + 64], v_ps[:])

        # ---------------- Phase C pools opened early so out_w prefetch overlaps B ---
        ctxC = ctx.enter_context(ExitStack())
        cpool = ctxC.enter_context(tc.tile_pool(name="cpool", bufs=1))
        wcolp = ctxC.enter_context(tc.tile_pool(name="wcolp", bufs=24))
        obuf = ctxC.enter_context(tc.tile_pool(name="obuf", bufs=4))
        oTown = cpool.tile([128, NKT * RPC], f16, tag="oTown")
        bias_sb = cpool.tile([128, E], f32, tag="bias")
        nc.sync.dma_start(bias_sb[:], biasr[:])
        wcols0 = []
        for kt in range(NKT):
            wc = wcolp.tile([128, 512], f16, tag="wc")
            nc.sync.dma_start(wc[:], outw[kt * 128 : (kt + 1) * 128, 0:512])
            wcols0.append(wc)
        wcols1 = []
        for kt in range(8):
            wc = wcolp.tile([128, 512], f16, tag="wc")
            nc.sync.dma_start(wc[:], outw[kt * 128 : (kt + 1) * 128, 512:1024])
            wcols1.append(wc)

        # ---------------- Phase B: attention (scoresT -> exp -> A@V) ----------------
        with ExitStack() as ctxB, nc.named_scope("phaseB"):
            expool = ctxB.enter_context(tc.tile_pool(name="expool", bufs=3))
            rpool = ctxB.enter_context(tc.tile_pool(name="rpool", bufs=3))
            rdram = ctxB.enter_context(tc.tile_pool(name="rdram", bufs=3, space="DRAM"))
            onorm = ctxB.enter_context(tc.tile_pool(name="onorm", bufs=4))
            psB = ctxB.enter_context(tc.tile_pool(name="psB", bufs=2, space="PSUM"))
            psO = ctxB.enter_context(tc.tile_pool(name="psO", bufs=2, space="PSUM"))

            for hp, qTt in ((0, qT0), (1, qT1)):
                a2a_buf = a2aA_in if hp == 0 else a2aB_in
                for j in range(NCORES):  # output row block = a2a destination core
                    b, qc = j // 4, j % 4
                    qs = slice(b * S + qc * 512, b * S + (qc + 1) * 512)
                    oT_ps = psO.tile([65, 1024], f32, tag="oT")
                    for kt in range(SKT):
                        ks = slice(b * S + kt * 128, b * S + (kt + 1) * 128)
                        sc = psB.tile([128, 1024], f32, tag="sc")
                        nc.tensor.matmul(
                            sc[:, 0:512], kT2[0:64, ks], qTt[0:64, qs],
                            start=True, stop=True,
                        )
                        nc.tensor.matmul(
                            sc[:, 512:1024], kT2[64:128, ks], qTt[64:128, qs],
                            start=True, stop=True,
                        )
                        ex = expool.tile([128, 1024], f16, tag="ex")
                        nc.scalar.activation(ex[:], sc[:], EXP, scale=0.125)
                        blk = (b * SKT + kt) * 65
                        st, sp = kt == 0, kt == SKT - 1
                        nc.tensor.matmul(
                            oT_ps[:, 0:512], vsb[:, blk : blk + 65], ex[:, 0:512],
                            start=st, stop=sp,
                        )
                        nc.tensor.matmul(
                            oT_ps[:, 512:1024], vsb[:, blk : blk + 65],
                            ex[:, 512:1024], start=st, stop=sp,
                        )
                    # normalize by the ones-row sum; write straight into a2a buffer
                    for hh in range(2):
                        hs = slice(hh * 512, (hh + 1) * 512)
                        rc = rpool.tile([1, 512], f32, tag="rc")
                        nc.vector.reciprocal(out=rc[:], in_=oT_ps[64:65, hs])
                        rcd = rdram.tile([1, 512], f32, tag="rcd")
                        nc.sync.dma_start(rcd[:], rc[:])
                        rb = rpool.tile([64, 512], f32, tag="rb")
                        nc.sync.dma_start(rb[:], rcd[0:1, :].to_broadcast((64, 512)))
                        on = onorm.tile([64, 512], f16, tag="on")
                        nc.vector.scalar_tensor_tensor(
                            out=on[:], in0=oT_ps[0:64, hs], scalar=1.0,
                            in1=rb[:], op0=MULT, op1=MULT,
                        )
                        nc.sync.dma_start(
                            a2a_buf[j, hh * 64 : (hh + 1) * 64, :], on[:]
                        )
                with nc.named_scope(f"a2a{hp}"):
                    nc.gpsimd.collective_compute(
                        "AllToAll",
                        mybir.AluOpType.bypass,
                        replica_groups=[list(range(NCORES))],
                        ins=[(a2aA_in if hp == 0 else a2aB_in).opt()],
                        outs=[(a2aA_out if hp == 0 else a2aB_out).opt()],
                    )

        # ---------------- Phase C: output projection for own row slice --------------
        with nc.named_scope("phaseC"):
            psC = ctxC.enter_context(tc.tile_pool(name="psC", bufs=3, space="PSUM"))

            for kt in range(NKT):
                src_t = a2aA_out if kt % 2 == 0 else a2aB_out
                nc.sync.dma_start(
                    oTown[:, kt * RPC : (kt + 1) * RPC], src_t[kt // 2, :, :]
                )

            for nch in range(4):
                ns = slice(nch * 512, (nch + 1) * 512)
                if nch == 0:
                    wcols = wcols0
                else:
                    wcols = wcols1 if nch == 1 else []
                    for kt in range(len(wcols), NKT):
                        wc = wcolp.tile([128, 512], f16, tag="wc")
                        nc.sync.dma_start(wc[:], outw[kt * 128 : (kt + 1) * 128, ns])
                        wcols.append(wc)
                for mt in range(4):
                    acc = psC.tile([128, 512], f32, tag="acc")
                    kt_order = list(range(0, NKT, 2)) + list(range(1, NKT, 2))
                    for i, kt in enumerate(kt_order):
                        nc.tensor.matmul(
                            acc[:],
                            oTown[:, kt * RPC + mt * 128 : kt * RPC + (mt + 1) * 128],
                            wcols[kt][:],
                            start=(i == 0), stop=(i == NKT - 1),
                        )
                    ob = obuf.tile([128, 512], f32, tag="ob")
                    nc.vector.scalar_tensor_tensor(
                        out=ob[:], in0=acc[:], scalar=1.0,
                        in1=bias_sb[:, ns], op0=MULT, op1=ADD,
                    )
                    nc.sync.dma_start(out[mt * 128 : (mt + 1) * 128, ns], ob[:])

    nc.finalize()
    return nc


def _prep_inputs(x, freqs_cos, freqs_sin, wq, wk, wv, out_w, out_b):
    x2 = np.ascontiguousarray(np.asarray(x, dtype=np.float32).reshape(ROWS, E))
    xT = np.ascontiguousarray(x2.T.astype(np.float16))

    cos = np.asarray(freqs_cos, dtype=np.float32).reshape(S, D // 2)
    sin = np.asarray(freqs_sin, dtype=np.float32).reshape(S, D // 2)
    cos_exp = np.repeat(cos.T, 2, axis=0)            # [64, S]
    sin_exp = np.repeat(sin.T, 2, axis=0)
    sin_exp[0::2] *= -1.0                            # -sin on even rows
    cosE = np.ascontiguousarray(np.tile(cos_exp, (2, 1)))  # [128, S]
    sinE = np.ascontiguousarray(np.tile(sin_exp, (2, 1)))

    perm = np.zeros((128, 128), dtype=np.float16)
    idx = np.arange(64)
    perm[2 * idx, 2 * idx + 1] = 1.0
    perm[2 * idx + 1, 2 * idx] = 1.0

    ident = np.tile(np.eye(64, dtype=np.float16), (2, 1))  # [128, 64]

    wq_f = np.asarray(wq, dtype=np.float32)
    wk_f = np.asarray(wk, dtype=np.float32)
    wv_f = np.asarray(wv, dtype=np.float32)
    outw_f = np.ascontiguousarray(np.asarray(out_w, dtype=np.float32).astype(np.float16))
    biasr = np.ascontiguousarray(
        np.tile(np.asarray(out_b, dtype=np.float32)[None, :], (128, 1))
    )

    in_maps = []
    for c in range(NCORES):
        wq_c = np.ascontiguousarray(
            wq_f[:, c * QCOLS : (c + 1) * QCOLS]
            .reshape(NKT, 128, QCOLS).transpose(1, 0, 2).reshape(128, NKT * QCOLS)
            .astype(np.float16)
        )
        wkv_c = np.ascontiguousarray(
            np.concatenate(
                [wk_f[:, c * 64 : (c + 1) * 64], wv_f[:, c * 64 : (c + 1) * 64]],
                axis=1,
            ).reshape(NKT, 128, 128).transpose(1, 0, 2).reshape(128, NKT * 128)
            .astype(np.float16)
        )
        in_maps.append(
            {
                "xT": xT, "wq": wq_c, "wkv": wkv_c, "cosE": cosE, "sinE": sinE,
                "perm": perm, "ident": ident, "outw": outw_f, "biasr": biasr,
                "vones": _VONES,
            }
        )
    return in_maps


def kernel(
    x, start_pos, freqs_cos, freqs_sin, wq, wk, wv, out_w, out_b,
    k_cache=None, v_cache=None, _trace=False, _trace_cores=None,
):
    from concourse.bass_utils import run_bass_kernel_spmd

    sp = int(np.asarray(start_pos))
    assert sp == 0, f"kernel specialized for start_pos=0, got {sp}"

    if "nc" not in _CACHE:
        _CACHE["nc"] = _build_module()
    nc = _CACHE["nc"]

    in_maps = _prep_inputs(x, freqs_cos, freqs_sin, wq, wk, wv, out_w, out_b)

    kwargs = {}
    if _trace:
        _install_ntff_hook()
        kwargs = {"trace": True, "trace_cores": _trace_cores}
    res = run_bass_kernel_spmd(nc, in_maps, list(range(NCORES)), **kwargs)

    full = np.concatenate([res.results[c]["out"] for c in range(NCORES)], axis=0)
    out = full.reshape(B, S, E).astype(np.float32)
    if _trace:
        return out, res
    return out


def _install_ntff_hook():
    """The agent image lacks antenv.axon_hooks; synthesize it so trace=True works."""
    import sys, types

    if "antenv.axon_hooks" in sys.modules:
        return
    try:
        from trn_agent_boot.trn_boot import _ntff_profile_via_ctypes

        hook = _ntff_profile_via_ctypes("/opt/axon/libaxon_pjrt.so")
    except Exception:
        hook = None
    mod = types.ModuleType("antenv.axon_hooks")
    mod.get_axon_ntff_profile_hook = lambda: hook
    sys.modules["antenv.axon_hooks"] = mod


# revision 26
# speedup vs baseline: 1.2230x; 1.0150x over previous
"""GQA attention block (B=2, S=2048, E=2048, H=32, HKV=8, D=64) on 8 trn2 cores.

Sharding: tensor-parallel over heads. Core c owns q-heads 4c..4c+3 and kv-head c.
Each core computes its heads' attention for ALL rows, then an AllToAll exchanges
head-blocks for row-blocks so each core runs the output projection for its own
512-row slice against the full out_w. Host concatenates row slices.

All matmuls run as float32r (TF32-like, full PE rate at N>=512). Softmax is
computed without max-subtraction (scores are O(4), exp cannot overflow), with
denominators obtained by augmenting V with a ones column.
"""

import numpy as np

B, S, E = 2, 2048, 2048
H, HKV, D = 32, 8, 64
NCORES = 8
ROWS = B * S              # 4096
RPC = ROWS // NCORES      # 512 output rows per core
HQ = H // NCORES          # 4 q heads per core
QCOLS = HQ * D            # 256
NCH = ROWS // 512         # 8 row chunks
NKT = E // 128            # 16 k-tiles over E
SKT = S // 128            # 16 key tiles per batch

_CACHE = {}
_VONES = np.zeros((128, B * (S // 128) * 128), dtype=np.float16)
_VONES[:, 64::128] = 1.0


def _build_module():
    from contextlib import ExitStack

    import concourse.tile as tile
    from concourse import bacc, mybir

    dt = mybir.dt
    f32, f32r, bf16 = dt.float32, dt.float32r, dt.bfloat16
    f16 = dt.float16
    EXP = mybir.ActivationFunctionType.Exp
    MULT = mybir.AluOpType.mult
    ADD = mybir.AluOpType.add

    nc = bacc.Bacc("TRN2", target_bir_lowering=False, debug=False, num_devices=NCORES)

    xT = nc.dram_tensor("xT", [E, ROWS], f16, kind="ExternalInput")
    wq = nc.dram_tensor("wq", [128, NKT * QCOLS], f16, kind="ExternalInput")
    wkv = nc.dram_tensor("wkv", [128, NKT * 128], f16, kind="ExternalInput")
    cosE = nc.dram_tensor("cosE", [128, S], f32, kind="ExternalInput")
    sinE = nc.dram_tensor("sinE", [128, S], f32, kind="ExternalInput")
    perm = nc.dram_tensor("perm", [128, 128], f16, kind="ExternalInput")
    ident = nc.dram_tensor("ident", [128, 64], f16, kind="ExternalInput")
    outw = nc.dram_tensor("outw", [E, E], f16, kind="ExternalInput")
    biasr = nc.dram_tensor("biasr", [128, E], f32, kind="ExternalInput")
    vones = nc.dram_tensor("vones", [128, B * SKT * 128], f16, kind="ExternalInput")
    out = nc.dram_tensor("out", [RPC, E], f32, kind="ExternalOutput")

    with tile.TileContext(nc) as tc, ExitStack() as ctx:
        persist = ctx.enter_context(tc.tile_pool(name="persist", bufs=1))
        dram = ctx.enter_context(tc.tile_pool(name="dram", bufs=1, space="DRAM"))

        qT0 = persist.tile([128, ROWS], f16, tag="qT0")  # heads 0,1 (local), D-major
        qT1 = persist.tile([128, ROWS], f16, tag="qT1")  # heads 2,3
        kT2 = persist.tile([128, ROWS], f16, tag="kT2")  # roped kT duplicated on 0:64 / 64:128
        vsb = persist.tile([128, B * SKT * 128], f16, tag="vsb")  # rows-major v + ones col
        perm_sb = persist.tile([128, 128], f16, tag="perm")
        ident_sb = persist.tile([128, 64], f16, tag="ident")

        nc.sync.dma_start(perm_sb[:], perm[:])
        nc.sync.dma_start(ident_sb[:], ident[:])
        # ones columns of the augmented-V tile (data columns overwritten later)
        nc.sync.dma_start(vsb[:], vones[:])

        a2aA_in = dram.tile([NCORES, 128, RPC], f16, tag="a2aA_in")
        a2aA_out = dram.tile([NCORES, 128, RPC], f16, tag="a2aA_out")
        a2aB_in = dram.tile([NCORES, 128, RPC], f16, tag="a2aB_in")
        a2aB_out = dram.tile([NCORES, 128, RPC], f16, tag="a2aB_out")

        # ---------------- Phase A: QKV projections + RoPE + V transpose -------------
        with ExitStack() as ctxA, nc.named_scope("phaseA"):
            wpool = ctxA.enter_context(tc.tile_pool(name="wpool", bufs=1))
            xpool = ctxA.enter_context(tc.tile_pool(name="xpool", bufs=20))
            cspool = ctxA.enter_context(tc.tile_pool(name="cspool", bufs=2))
            tmpA = ctxA.enter_context(tc.tile_pool(name="tmpA", bufs=2))
            psA1 = ctxA.enter_context(tc.tile_pool(name="psA1", bufs=2, space="PSUM"))
            psA2 = ctxA.enter_context(tc.tile_pool(name="psA2", bufs=1, space="PSUM"))

            wq_sb = wpool.tile([128, NKT * QCOLS], f16, tag="wq")
            wkv_sb = wpool.tile([128, NKT * 128], f16, tag="wkv")
            nc.sync.dma_start(wq_sb[:], wq[:])
            nc.sync.dma_start(wkv_sb[:], wkv[:])

            for chp in range(NCH // 2):
                ps1k = slice(chp * 1024, (chp + 1) * 1024)
                xts = []
                for kt in range(NKT):
                    xt = xpool.tile([128, 1024], f16, tag="xt")
                    nc.sync.dma_start(xt[:], xT[kt * 128 : (kt + 1) * 128, ps1k])
                    xts.append(xt)
                for sub in range(2):
                    ch = chp * 2 + sub
                    cs = slice(ch * 512, (ch + 1) * 512)
                    ss = slice(sub * 512, (sub + 1) * 512)
                    q0_ps = psA1.tile([128, 512], f32, tag="q0")
                    q1_ps = psA1.tile([128, 512], f32, tag="q1")
                    kv_ps = psA1.tile([128, 512], f32, tag="kv")
                    for kt in range(NKT):
                        st, sp = kt == 0, kt == NKT - 1
                        xs = xts[kt][:, ss]
                        wqk = wq_sb[:, kt * QCOLS : kt * QCOLS + 128]
                        wqk2 = wq_sb[:, kt * QCOLS + 128 : kt * QCOLS + 256]
                        nc.tensor.matmul(q0_ps[:], wqk, xs, start=st, stop=sp)
                        nc.tensor.matmul(q1_ps[:], wqk2, xs, start=st, stop=sp)
                        nc.tensor.matmul(
                            kv_ps[:], wkv_sb[:, kt * 128 : (kt + 1) * 128], xs,
                            start=st, stop=sp,
                        )

                    # PSUM -> SBUF (rounds to f32r)
                    nc.scalar.copy(qT0[:, cs], q0_ps[:])
                    nc.scalar.copy(qT1[:, cs], q1_ps[:])
                    nc.scalar.copy(kT2[0:64, cs], kv_ps[0:64, :])
                    vtt = tmpA.tile([128, 512], f16, tag="vtt")
                    nc.scalar.copy(vtt[64:128, :], kv_ps[64:128, :])

                    # RoPE: t = t*cosE + (perm @ t)*sinE   (in place)
                    scs = slice((ch % 4) * 512, (ch % 4 + 1) * 512)  # pos = row % S
                    cos_sb = cspool.tile([128, 512], f32, tag="cos")
                    sin_sb = cspool.tile([128, 512], f32, tag="sin")
                    nc.sync.dma_start(cos_sb[:], cosE[:, scs])
                    nc.sync.dma_start(sin_sb[:], sinE[:, scs])
                    for t, p in ((qT0, 128), (qT1, 128), (kT2, 64)):
                        rot_ps = psA2.tile([128, 512], f32, tag="rot")
                        nc.tensor.matmul(
                            rot_ps[0:p, :], perm_sb[0:p, 0:p], t[0:p, cs],
                            start=True, stop=True,
                        )
                        tmp = tmpA.tile([128, 512], f32, tag="ropetmp")
                        nc.vector.scalar_tensor_tensor(
                            out=tmp[0:p, :], in0=rot_ps[0:p, :], scalar=1.0,
                            in1=sin_sb[0:p, :], op0=MULT, op1=MULT,
                        )
                        nc.vector.scalar_tensor_tensor(
                            out=t[0:p, cs], in0=t[0:p, cs], scalar=1.0,
                            in1=cos_sb[0:p, :], op0=MULT, op1=MULT,
                        )
                        nc.vector.scalar_tensor_tensor(
                            out=t[0:p, cs], in0=t[0:p, cs], scalar=1.0,
                            in1=tmp[0:p, :], op0=MULT, op1=ADD,
                        )
                    # duplicate roped k on partitions 64:128 (for row-group packing)
                    nc.sync.dma_start(kT2[64:128, cs], kT2[0:64, cs])

                    # V transpose: [64,512] (keys on free) -> 4x [128,64] rows-major
                    b = ch // 4
                    for j in range(4):
                        kt_key = (ch % 4) * 4 + j
                        v_ps = psA2.tile([128, 64], f16, tag="vps")
                        nc.tensor.transpose(
                            v_ps[:], vtt[64:128, j * 128 : (j + 1) * 128],
                            ident_sb[64:128, :],
                        )
                        blk = (b * SKT + kt_key) * 128
                        nc.vector.tensor_copy(vsb[:, blk : blk + 64], v_ps[:])

        # ---------------- Phase C pools opened early so out_w prefetch overlaps B ---
        ctxC = ctx.enter_context(ExitStack())
        cpool = ctxC.enter_context(tc.tile_pool(name="cpool", bufs=1))
        wcolp = ctxC.enter_context(tc.tile_pool(name="wcolp", bufs=24))
        obuf = ctxC.enter_context(tc.tile_pool(name="obuf", bufs=4))
        oTown = cpool.tile([128, NKT * RPC], f16, tag="oTown")
        bias_sb = cpool.tile([128, E], f32, tag="bias")
        nc.sync.dma_start(bias_sb[:], biasr[:])
        wcols0 = []
        for kt in range(NKT):
            wc = wcolp.tile([128, 512], f16, tag="wc")
            nc.sync.dma_start(wc[:], outw[kt * 128 : (kt + 1) * 128, 0:512])
            wcols0.append(wc)
        wcols1 = []
        for kt in range(8):
            wc = wcolp.tile([128, 512], f16, tag="wc")
            nc.sync.dma_start(wc[:], outw[kt * 128 : (kt + 1) * 128, 512:1024])
            wcols1.append(wc)

        # ---------------- Phase B: attention (scoresT -> exp -> A@V) ----------------
        with ExitStack() as ctxB, nc.named_scope("phaseB"):
            expool = ctxB.enter_context(tc.tile_pool(name="expool", bufs=3))
            rpool = ctxB.enter_context(tc.tile_pool(name="rpool", bufs=3))
            rdram = ctxB.enter_context(tc.tile_pool(name="rdram", bufs=3, space="DRAM"))
            onorm = ctxB.enter_context(tc.tile_pool(name="onorm", bufs=4))
            psB = ctxB.enter_context(tc.tile_pool(name="psB", bufs=2, space="PSUM"))
            psO = ctxB.enter_context(tc.tile_pool(name="psO", bufs=2, space="PSUM"))

            for hp, qTt in ((0, qT0), (1, qT1)):
                a2a_buf = a2aA_in if hp == 0 else a2aB_in
                for j in range(NCORES):  # output row block = a2a destination core
                    b, qc = j // 4, j % 4
                    qs = slice(b * S + qc * 512, b * S + (qc + 1) * 512)
                    oT_ps = psO.tile([128, 1024], f32, tag="oT")
                    for kt in range(SKT):
                        ks = slice(b * S + kt * 128, b * S + (kt + 1) * 128)
                        sc = psB.tile([128, 1024], f32, tag="sc")
                        nc.tensor.matmul(
                            sc[:, 0:512], kT2[0:64, ks], qTt[0:64, qs],
                            start=True, stop=True,
                        )
                        nc.tensor.matmul(
                            sc[:, 512:1024], kT2[64:128, ks], qTt[64:128, qs],
                            start=True, stop=True,
                        )
                        ex = expool.tile([128, 1024], f16, tag="ex")
                        nc.scalar.activation(ex[:], sc[:], EXP, scale=0.125)
                        blk = (b * SKT + kt) * 128
                        st, sp = kt == 0, kt == SKT - 1
                        nc.tensor.matmul(
                            oT_ps[:, 0:512], vsb[:, blk : blk + 128], ex[:, 0:512],
                            start=st, stop=sp,
                        )
                        nc.tensor.matmul(
                            oT_ps[:, 512:1024], vsb[:, blk : blk + 128],
                            ex[:, 512:1024], start=st, stop=sp,
                        )
                    # normalize by the ones-row sum; write straight into a2a buffer
                    for hh in range(2):
                        hs = slice(hh * 512, (hh + 1) * 512)
                        rc = rpool.tile([1, 512], f32, tag="rc")
                        nc.vector.reciprocal(out=rc[:], in_=oT_ps[64:65, hs])
                        rcd = rdram.tile([1, 512], f32, tag="rcd")
                        nc.sync.dma_start(rcd[:], rc[:])
                        rb = rpool.tile([64, 512], f32, tag="rb")
                        nc.sync.dma_start(rb[:], rcd[0:1, :].to_broadcast((64, 512)))
                        on = onorm.tile([64, 512], f16, tag="on")
                        nc.vector.scalar_tensor_tensor(
                            out=on[:], in0=oT_ps[0:64, hs], scalar=1.0,
                            in1=rb[:], op0=MULT, op1=MULT,
                        )
                        nc.sync.dma_start(
                            a2a_buf[j, hh * 64 : (hh + 1) * 64, :], on[:]
                        )
                with nc.named_scope(f"a2a{hp}"):
                    nc.gpsimd.collective_compute(
                        "AllToAll",
                        mybir.AluOpType.bypass,
                        replica_groups=[list(range(NCORES))],
                        ins=[(a2aA_in if hp == 0 else a2aB_in).opt()],
                        outs=[(a2aA_out if hp == 0 else a2aB_out).opt()],
                    )

        # ---------------- Phase C: output projection for own row slice --------------
        with nc.named_scope("phaseC"):
            psC = ctxC.enter_context(tc.tile_pool(name="psC", bufs=3, space="PSUM"))

            for kt in range(NKT):
                src_t = a2aA_out if kt % 2 == 0 else a2aB_out
                nc.sync.dma_start(
                    oTown[:, kt * RPC : (kt + 1) * RPC], src_t[kt // 2, :, :]
                )

            for nch in range(4):
                ns = slice(nch * 512, (nch + 1) * 512)
                if nch == 0:
                    wcols = wcols0
                else:
                    wcols = wcols1 if nch == 1 else []
                    for kt in range(len(wcols), NKT):
                        wc = wcolp.tile([128, 512], f16, tag="wc")
                        nc.sync.dma_start(wc[:], outw[kt * 128 : (kt + 1) * 128, ns])
                        wcols.append(wc)
                for mt in range(4):
                    acc = psC.tile([128, 512], f32, tag="acc")
                    kt_order = list(range(0, NKT, 2)) + list(range(1, NKT, 2))
                    for i, kt in enumerate(kt_order):
                        nc.tensor.matmul(
                            acc[:],
                            oTown[:, kt * RPC + mt * 128 : kt * RPC + (mt + 1) * 128],
                            wcols[kt][:],
                            start=(i == 0), stop=(i == NKT - 1),
                        )
                    ob = obuf.tile([128, 512], f32, tag="ob")
                    nc.vector.scalar_tensor_tensor(
                        out=ob[:], in0=acc[:], scalar=1.0,
                        in1=bias_sb[:, ns], op0=MULT, op1=ADD,
                    )
                    nc.sync.dma_start(out[mt * 128 : (mt + 1) * 128, ns], ob[:])

    nc.finalize()
    return nc


def _prep_inputs(x, freqs_cos, freqs_sin, wq, wk, wv, out_w, out_b):
    x2 = np.ascontiguousarray(np.asarray(x, dtype=np.float32).reshape(ROWS, E))
    xT = np.ascontiguousarray(x2.T.astype(np.float16))

    cos = np.asarray(freqs_cos, dtype=np.float32).reshape(S, D // 2)
    sin = np.asarray(freqs_sin, dtype=np.float32).reshape(S, D // 2)
    cos_exp = np.repeat(cos.T, 2, axis=0)            # [64, S]
    sin_exp = np.repeat(sin.T, 2, axis=0)
    sin_exp[0::2] *= -1.0                            # -sin on even rows
    cosE = np.ascontiguousarray(np.tile(cos_exp, (2, 1)))  # [128, S]
    sinE = np.ascontiguousarray(np.tile(sin_exp, (2, 1)))

    perm = np.zeros((128, 128), dtype=np.float16)
    idx = np.arange(64)
    perm[2 * idx, 2 * idx + 1] = 1.0
    perm[2 * idx + 1, 2 * idx] = 1.0

    ident = np.tile(np.eye(64, dtype=np.float16), (2, 1))  # [128, 64]

    wq_f = np.asarray(wq, dtype=np.float32)
    wk_f = np.asarray(wk, dtype=np.float32)
    wv_f = np.asarray(wv, dtype=np.float32)
    outw_f = np.ascontiguousarray(np.asarray(out_w, dtype=np.float32).astype(np.float16))
    biasr = np.ascontiguousarray(
        np.tile(np.asarray(out_b, dtype=np.float32)[None, :], (128, 1))
    )

    in_maps = []
    for c in range(NCORES):
        wq_c = np.ascontiguousarray(
            wq_f[:, c * QCOLS : (c + 1) * QCOLS]
            .reshape(NKT, 128, QCOLS).transpose(1, 0, 2).reshape(128, NKT * QCOLS)
            .astype(np.float16)
        )
        wkv_c = np.ascontiguousarray(
            np.concatenate(
                [wk_f[:, c * 64 : (c + 1) * 64], wv_f[:, c * 64 : (c + 1) * 64]],
                axis=1,
            ).reshape(NKT, 128, 128).transpose(1, 0, 2).reshape(128, NKT * 128)
            .astype(np.float16)
        )
        in_maps.append(
            {
                "xT": xT, "wq": wq_c, "wkv": wkv_c, "cosE": cosE, "sinE": sinE,
                "perm": perm, "ident": ident, "outw": outw_f, "biasr": biasr,
                "vones": _VONES,
            }
        )
    return in_maps


def kernel(
    x, start_pos, freqs_cos, freqs_sin, wq, wk, wv, out_w, out_b,
    k_cache=None, v_cache=None, _trace=False, _trace_cores=None,
):
    from concourse.bass_utils import run_bass_kernel_spmd

    sp = int(np.asarray(start_pos))
    assert sp == 0, f"kernel specialized for start_pos=0, got {sp}"

    if "nc" not in _CACHE:
        _CACHE["nc"] = _build_module()
    nc = _CACHE["nc"]

    in_maps = _prep_inputs(x, freqs_cos, freqs_sin, wq, wk, wv, out_w, out_b)

    kwargs = {}
    if _trace:
        _install_ntff_hook()
        kwargs = {"trace": True, "trace_cores": _trace_cores}
    res = run_bass_kernel_spmd(nc, in_maps, list(range(NCORES)), **kwargs)

    full = np.concatenate([res.results[c]["out"] for c in range(NCORES)], axis=0)
    out = full.reshape(B, S, E).astype(np.float32)
    if _trace:
        return out, res
    return out


def _install_ntff_hook():
    """The agent image lacks antenv.axon_hooks; synthesize it so trace=True works."""
    import sys, types

    if "antenv.axon_hooks" in sys.modules:
        return
    try:
        from trn_agent_boot.trn_boot import _ntff_profile_via_ctypes

        hook = _ntff_profile_via_ctypes("/opt/axon/libaxon_pjrt.so")
    except Exception:
        hook = None
    mod = types.ModuleType("antenv.axon_hooks")
    mod.get_axon_ntff_profile_hook = lambda: hook
    sys.modules["antenv.axon_hooks"] = mod
